# revision 1
# baseline (speedup 1.0000x reference)
"""Trainium2 Bass kernel for nn_MIGAModel (moe_routing).

Strategy (pure data parallel over the stock axis N, 8 cores):
 - Host pre-transposes each core's x shard to xT [T*D, N/8] so the
   contraction dim lands on SBUF partitions with large contiguous DMAs.
 - T-layout on chip: features on partitions, rows on the free axis.
 - Router: hT[128, rows] accumulated over 75 K-chunks into 5 PSUM banks
   (500 rows each, all 2500 shard rows resident at once).
 - Per-group attention is expressed as 128x128 matmuls against
   host-prebuilt block-diagonal / permutation / replication matrices,
   plus a handful of full-width DVE elementwise ops.  Softmax over the
   4-wide axis needs no max-subtraction (scores are O(0.1)).
 - Exact top-2 gating: PE transposes of h, free-axis reduce_max twice,
   exact fp32 PE broadcast of the per-row threshold, weighted sum via
   ones-matmuls.
"""
import sys
import numpy as np

for _p in ("/opt/trn_rl_repo",):
    if _p not in sys.path:
        sys.path.insert(0, _p)

import concourse.bass as bass
import concourse.tile as tile
from concourse import bacc, mybir
from concourse.bass_utils import run_bass_kernel_spmd

F32 = mybir.dt.float32
F32R = mybir.dt.float32r
BF16 = mybir.dt.bfloat16

N, T, D = 20000, 60, 158
TD = T * D                      # 9480
G, E, H, DH, GE = 8, 16, 4, 4, 128
NCORES = 8
NSH = N // NCORES               # 2500 rows per core
KT = (TD + 127) // 128          # 75 K-chunks
TDP = KT * 128                  # 9600 padded contraction dim
CH = 500                        # rows per processing chunk (1 PSUM bank)
NCH = NSH // CH                 # 5 chunks

# packed matrix indices (each a [128,128] block in the "mats" input)
M_WET, M_AQ = 0, 1
M_AK0, M_AV0 = 2, 6             # 4 each
M_MS0 = 10                      # 4
M_MDEN = 14
M_MER0 = 15                     # 4
M_AO = 19
M_IDT = 20
M_ONES = 21
NMATS = 22

# bias pack columns
B_BE, B_BQ, B_BK0, B_BV0, B_BO, B_BR = 0, 1, 2, 6, 10, 11
NBIAS = 16


def build_consts(Wr, br, We, be, Wq, bq, Wk, bk, Wv, bv, Wo, bo):
    """Host-side packed constants. Returns (wr_pad, mats, biasp)."""
    f32 = np.float32
    Wr = np.asarray(Wr, f32)
    br = np.asarray(br, f32)
    We = np.asarray(We, f32)
    be = np.asarray(be, f32)
    Wq = np.asarray(Wq, f32)
    bq = np.asarray(bq, f32)
    Wk = np.asarray(Wk, f32)
    bk = np.asarray(bk, f32)
    Wv = np.asarray(Wv, f32)
    bv = np.asarray(bv, f32)
    Wo = np.asarray(Wo, f32)
    bo = np.asarray(bo, f32)

    # router weight, K-padded; bias folded in as one extra contraction row
    # on a constant-1 input column is NOT used -- br added via ACT bias.
    wr_pad = np.zeros((TDP, GE), f32)
    wr_pad[:TD] = Wr

    mats = np.zeros((NMATS, GE, GE), f32)
    biasp = np.zeros((GE, NBIAS), f32)

    mats[M_WET] = np.transpose(We, (2, 0, 1)).reshape(GE, GE)
    biasp[:, B_BE] = be.reshape(GE)
    biasp[:, B_BO] = bo.reshape(GE)

    d_ = np.arange(DH)
    for g in range(G):
        for h in range(H):
            for d in range(DH):
                p = d * 32 + g * 4 + h
                mats[M_AQ, g * 16:(g + 1) * 16, p] = Wq[g, h * 4 + d, :]
                biasp[p, B_BQ] = bq[g, h * 4 + d]
            for e in range(DH):
                ps = d_ * 32 + g * 4 + h
                for p in ps:
                    mats[M_AK0 + e, g * 16:(g + 1) * 16, p] = Wk[g, h * 4 + e, :]
                    mats[M_AV0 + e, g * 16:(g + 1) * 16, p] = Wv[g, h * 4 + e, :]
                    biasp[p, B_BK0 + e] = bk[g, h * 4 + e]
                    biasp[p, B_BV0 + e] = bv[g, h * 4 + e]
    for e in range(DH):
        for d in range(DH):
            for g in range(G):
                for h in range(H):
                    mats[M_MS0 + e, d * 32 + g * 4 + h, e * 32 + d * 8 + g] = 1.0
                    mats[M_MDEN, e * 32 + d * 8 + g, d * 32 + g * 4 + h] = 1.0
                    mats[M_MER0 + e, e * 32 + d * 8 + g, d * 32 + g * 4 + h] = 1.0
    for g in range(G):
        for f in range(E):
            for h in range(H):
                for d in range(DH):
                    mats[M_AO, d * 32 + g * 4 + h, g * 16 + f] = Wo[g, f, h * 4 + d]
    mats[M_IDT] = np.eye(GE, dtype=f32)
    mats[M_ONES] = 1.0

    # [128, NMATS*128] column-packed
    mats_packed = np.ascontiguousarray(np.transpose(mats, (1, 0, 2)).reshape(GE, NMATS * GE))
    return wr_pad, mats_packed, biasp


def build_kernel():
    """Trace the Bass/Tile kernel; returns the compiled Bacc."""
    nc = bacc.Bacc("TRN2", target_bir_lowering=False, debug=False,
                   num_devices=NCORES)

    xt_d = nc.dram_tensor("xt", [TDP, NSH], F32, kind="ExternalInput").ap()
    wr_d = nc.dram_tensor("wr", [TDP, GE], F32, kind="ExternalInput").ap()
    mats_d = nc.dram_tensor("mats", [GE, NMATS * GE], F32, kind="ExternalInput").ap()
    bias_d = nc.dram_tensor("bias", [GE, NBIAS], F32, kind="ExternalInput").ap()
    out_d = nc.dram_tensor("out", [1, NSH], F32, kind="ExternalOutput").ap()

    with tile.TileContext(nc) as tc:
        with (
            tc.tile_pool(name="consts", bufs=1) as consts,
            tc.tile_pool(name="xts", bufs=3) as xts,
            tc.tile_pool(name="work", bufs=2) as work,
            tc.tile_pool(name="ps", bufs=8, space="PSUM") as ptp,
        ):
            # ---- constants to SBUF ----
            wr_sb = consts.tile([128, KT, GE], F32, tag="wr")
            nc.sync.dma_start(out=wr_sb, in_=wr_d.rearrange("(c p) m -> p c m", p=128))
            mats_sb = consts.tile([GE, NMATS * GE], F32, tag="mats")
            nc.sync.dma_start(out=mats_sb, in_=mats_d)
            bias_sb = consts.tile([GE, NBIAS], F32, tag="bias")
            nc.sync.dma_start(out=bias_sb, in_=bias_d)
            pred_sb = consts.tile([1, NSH], F32, tag="pred")

            def mat(i, r=True):
                ap = mats_sb[:, i * GE:(i + 1) * GE]
                return ap

            def bcol(i):
                return bias_sb[:, i:i + 1]

            idt = mat(M_IDT, r=False)
            ones = mat(M_ONES, r=False)

            # ---- router: hT accumulation over K-chunks ----
            hps = [ptp.tile([GE, CH], F32, tag="pt", name=f"hps{c}")
                   for c in range(NCH)]
            for t in range(KT):
                xt_t = xts.tile([128, NSH], F32, tag="xt")
                nc.sync.dma_start(out=xt_t, in_=xt_d[t * 128:(t + 1) * 128, :])
                for c in range(NCH):
                    sl = slice(c * CH, (c + 1) * CH)
                    nc.tensor.matmul(hps[c][:, :], lhsT=wr_sb[:, t, :],
                                     rhs=xt_t[:, sl],
                                     start=(t == 0), stop=(t == KT - 1))

            # ---- per-chunk post processing ----
            for c in range(NCH):
                # h to SBUF, + router bias br is folded on host into... no:
                # br is added here via activation bias (per-partition [128,1])
                h_sb = work.tile([GE, CH], F32, tag="h")
                nc.scalar.activation(h_sb, hps[c][:, :],
                                     mybir.ActivationFunctionType.Identity,
                                     bias=bcol(B_BR), scale=1.0)

                # --- exact top-2 threshold (second max per row) ---
                m2t_ps = ptp.tile([1, CH], F32, tag="pt")
                off = 0
                while off < CH:
                    cs = min(128, CH - off)
                    tr_ps = ptp.tile([128, 128], F32, tag="pt")
                    nc.tensor.transpose(tr_ps[:cs, :GE], h_sb[:, off:off + cs], idt)
                    mx1 = work.tile([128, 1], F32, tag="mx1")
                    nc.vector.reduce_max(mx1[:cs], tr_ps[:cs, :GE],
                                         axis=mybir.AxisListType.X)
                    eqm = work.tile([128, GE], F32, tag="eqm")
                    nc.vector.tensor_scalar(eqm[:cs], tr_ps[:cs, :GE], mx1[:cs],
                                            None, op0=mybir.AluOpType.is_ge)
                    hm = work.tile([128, GE], F32, tag="hm")
                    nc.vector.scalar_tensor_tensor(
                        hm[:cs], in0=eqm[:cs], scalar=-1e30, in1=tr_ps[:cs, :GE],
                        op0=mybir.AluOpType.mult, op1=mybir.AluOpType.add)
                    mx2 = work.tile([128, 1], F32, tag="mx2")
                    nc.vector.reduce_max(mx2[:cs], hm[:cs],
                                         axis=mybir.AxisListType.X)
                    # put the per-row threshold back into row-vector layout
                    nc.tensor.matmul(m2t_ps[0:1, off:off + cs], lhsT=mx2[:cs],
                                     rhs=idt[:cs, :cs], start=True, stop=True)
                    off += cs
                m2t_sb = work.tile([1, CH], F32, tag="m2t")
                nc.scalar.activation(m2t_sb, m2t_ps[:, :],
                                     mybir.ActivationFunctionType.Copy)
                # exact broadcast down 128 partitions (K=1 matmul, 1.0*v)
                m2b_ps = ptp.tile([GE, CH], F32, tag="pt")
                nc.tensor.matmul(m2b_ps[:, :], lhsT=ones[0:1, :],
                                 rhs=m2t_sb[0:1, :], start=True, stop=True)
                mask = work.tile([GE, CH], F32, tag="mask")
                nc.vector.tensor_tensor(mask, h_sb, m2b_ps[:, :],
                                        op=mybir.AluOpType.is_ge)
                eh = work.tile([GE, CH], F32, tag="eh")
                nc.scalar.activation(eh, h_sb, mybir.ActivationFunctionType.Exp)
                m1 = work.tile([GE, CH], F32, tag="m1")
                nc.vector.tensor_mul(m1, eh, mask)

                # --- experts + attention ---
                eo_ps = ptp.tile([GE, CH], F32, tag="pt")
                nc.tensor.matmul(eo_ps[:, :], lhsT=mat(M_WET),
                                 rhs=h_sb, start=True, stop=True)
                eo_sb = work.tile([GE, CH], F32, tag="eo")
                nc.vector.tensor_scalar_add(eo_sb, eo_ps[:, :], bcol(B_BE))
                eo_r = eo_sb

                q_ps = ptp.tile([GE, CH], F32, tag="pt")
                nc.tensor.matmul(q_ps[:, :], lhsT=mat(M_AQ), rhs=eo_r,
                                 start=True, stop=True)
                qt_sb = work.tile([GE, CH], F32, tag="qt")
                nc.vector.tensor_scalar_add(qt_sb, q_ps[:, :], bcol(B_BQ))

                sc_ps = ptp.tile([GE, CH], F32, tag="pt")
                for e in range(DH):
                    kr_ps = ptp.tile([GE, CH], F32, tag="pt")
                    nc.tensor.matmul(kr_ps[:, :], lhsT=mat(M_AK0 + e), rhs=eo_r,
                                     start=True, stop=True)
                    kr_sb = work.tile([GE, CH], F32, tag="kr")
                    nc.vector.tensor_scalar_add(kr_sb, kr_ps[:, :], bcol(B_BK0 + e))
                    pe_sb = work.tile([GE, CH], F32, tag="pe")
                    nc.vector.tensor_mul(pe_sb, qt_sb, kr_sb)
                    nc.tensor.matmul(sc_ps[:, :], lhsT=mat(M_MS0 + e),
                                     rhs=pe_sb,
                                     start=(e == 0), stop=(e == DH - 1))
                es_sb = work.tile([GE, CH], F32, tag="es")
                nc.scalar.activation(es_sb, sc_ps[:, :],
                                     mybir.ActivationFunctionType.Exp, scale=0.5)
                es_r = es_sb

                den_ps = ptp.tile([GE, CH], F32, tag="pt")
                nc.tensor.matmul(den_ps[:, :], lhsT=mat(M_MDEN), rhs=es_r,
                                 start=True, stop=True)
                drec = work.tile([GE, CH], F32, tag="drec")
                nc.vector.reciprocal(drec, den_ps[:, :])

                att = work.tile([GE, CH], F32, tag="att")
                for e in range(DH):
                    vr_ps = ptp.tile([GE, CH], F32, tag="pt")
                    nc.tensor.matmul(vr_ps[:, :], lhsT=mat(M_AV0 + e), rhs=eo_r,
                                     start=True, stop=True)
                    vr_sb = work.tile([GE, CH], F32, tag="vr")
                    nc.vector.tensor_scalar_add(vr_sb, vr_ps[:, :], bcol(B_BV0 + e))
                    er_ps = ptp.tile([GE, CH], F32, tag="pt")
                    nc.tensor.matmul(er_ps[:, :], lhsT=mat(M_MER0 + e), rhs=es_r,
                                     start=True, stop=True)
                    if e == 0:
                        nc.vector.tensor_mul(att, er_ps[:, :], vr_sb)
                    else:
                        pr = work.tile([GE, CH], F32, tag="pr")
                        nc.vector.tensor_mul(pr, er_ps[:, :], vr_sb)
                        nc.vector.tensor_add(att, att, pr)
                nc.vector.tensor_mul(att, att, drec)

                ao_ps = ptp.tile([GE, CH], F32, tag="pt")
                nc.tensor.matmul(ao_ps[:, :], lhsT=mat(M_AO),
                                 rhs=att, start=True, stop=True)
                aout = work.tile([GE, CH], F32, tag="aout")
                nc.vector.tensor_scalar_add(aout, ao_ps[:, :], bcol(B_BO))

                # --- weighted combine ---
                num = work.tile([GE, CH], F32, tag="num")
                nc.vector.tensor_mul(num, m1, aout)
                dens_ps = ptp.tile([1, CH], F32, tag="pt")
                nc.tensor.matmul(dens_ps[:, :], lhsT=ones[:, 0:1], rhs=m1,
                                 start=True, stop=True)
                nums_ps = ptp.tile([1, CH], F32, tag="pt")
                nc.tensor.matmul(nums_ps[:, :], lhsT=ones[:, 0:1], rhs=num,
                                 start=True, stop=True)
                rden = work.tile([1, CH], F32, tag="rden")
                nc.vector.reciprocal(rden, dens_ps[:, :])
                nc.vector.tensor_mul(pred_sb[0:1, c * CH:(c + 1) * CH],
                                     nums_ps[:, :], rden)

            nc.sync.dma_start(out=out_d, in_=pred_sb[:, :])

    nc.compile()
    return nc


_NC_CACHE = None
LAST_RESULTS = None


def kernel(x, Wr, br, We, be, Wq, bq, Wk, bk, Wv, bv, Wo, bo):
    global _NC_CACHE, LAST_RESULTS
    f32 = np.float32
    x = np.asarray(x, f32)

    wr_pad, mats_packed, biasp = build_consts(
        Wr, br, We, be, Wq, bq, Wk, bk, Wv, bv, Wo, bo)
    biasp[:, B_BR] = np.asarray(br, f32)

    if _NC_CACHE is None:
        _NC_CACHE = build_kernel()
    nc = _NC_CACHE

    in_maps = []
    for c in range(NCORES):
        xs = x[c * NSH:(c + 1) * NSH].reshape(NSH, TD)
        xt = np.zeros((TDP, NSH), f32)
        xt[:TD] = np.ascontiguousarray(xs.T)
        in_maps.append({"xt": xt, "wr": wr_pad, "mats": mats_packed,
                        "bias": biasp})

    res = run_bass_kernel_spmd(nc, in_maps, list(range(NCORES)))
    LAST_RESULTS = res
    out = np.concatenate([res.results[c]["out"].reshape(NSH)
                          for c in range(NCORES)])
    return out.astype(f32)



# revision 6
# speedup vs baseline: 1.4809x; 1.4809x over previous
"""Trainium2 Bass kernel for nn_MIGAModel (moe_routing).

Pure data parallel over the stock axis N (8 cores, 2500 rows each).

Router precision scheme (the top-2 gating is discontinuous in the router
logits h, so h must be fp32-accurate to ~1e-5; plain fp16/bf16/fp32r
inputs all flip expert selections and fail the 2e-2 gate):
    x  = a + b      a = fp16(x),  b8 = fp8e4m3(b * 2048)
    Wr = c + d      c = fp16(Wr), d16 = fp16(d * 2048)
    h  = a@c  +  (a@d16 + b8@c8) / 2048        (c8 = fp8(c))
Three 1-cycle/row PE passes (fp16, fp16, fp8-moving x fp8-stationary),
two PSUM banks (main, aux), one DVE op to combine.  The router bias br
rides in as an extra contraction row (a row of ones in `a`, br split
across c/d16), so selection sees the exact biased logits.  delta-h is
~1e-5 -> end-to-end rel err ~3e-3 (selection flips dominate; measured).

DMA: a is 2 B/elem, b8 1 B/elem -> 72 MB/core vs 95 MB for fp32.
Post-processing (experts + inner-group attention as 128x128
block-diagonal matmuls) runs on bf16 operands (1 cyc/row), biases are
folded into ACT-engine PSUM->SBUF moves, top-2 stays in fp32.
Row-chunk pipeline: router(c+1) is emitted before post(c) so PSUM-bank
or DVE waits in the post chain never stall the router matmul stream.
Output DMAs issue from the ACT queue to keep the SP queue (x tiles)
free-running.
"""
import sys
import numpy as np

for _p in ("/opt/trn_rl_repo",):
    if _p not in sys.path:
        sys.path.insert(0, _p)

import ml_dtypes

import concourse.bass as bass
import concourse.tile as tile
from concourse import bacc, mybir
from concourse.bass_utils import run_bass_kernel_spmd

F32 = mybir.dt.float32
F16 = mybir.dt.float16
F8 = mybir.dt.float8e4
BF16 = mybir.dt.bfloat16

N, T, D = 20000, 60, 158
TD = T * D                      # 9480
G, E, H, DH, GE = 8, 16, 4, 4, 128
NCORES = 8
NSH = N // NCORES               # 2500 rows per core
KT = 75                         # fp16 K-chunks of 128 (9600 padded, row 9480 = ones)
TDP = KT * 128                  # 9600
KT2 = 38                        # fp8 K-pairs (9728 padded)
TDP8 = KT2 * 256                # 9728
CH = 500                        # rows per compute chunk (1 PSUM bank)
NCH = NSH // CH                 # 5 chunks
NQ = 5                          # a-tile K-groups per chunk (15 K-chunks each)
KQ = KT // NQ                   # 15
RS = 2048.0                     # residual scale (2**11)

# bf16 packed matrix indices ([128,128] blocks in "mats16")
M_WET, M_AQ = 0, 1
M_AK0, M_AV0 = 2, 6             # 4 each
M_MS0 = 10                      # 4
M_MDEN = 14
M_MER0 = 15                     # 4
M_AO = 19
M_IDT = 20
M_ONES = 21
NM16 = 22

# fp32 packed matrices ([128,128] blocks in "mats32"): identity, ones
M32_IDT, M32_ONES = 0, 1
NM32 = 2

# bias pack columns (fp32)
B_BE, B_BQ, B_BK0, B_BV0, B_BO = 0, 1, 2, 6, 10
NBIAS = 11


def build_consts(Wr, br, We, be, Wq, bq, Wk, bk, Wv, bv, Wo, bo):
    """Host-side packed constants.

    Returns (c16, d16s, c8, mats16, mats32, biasp):
      c16  [128, KT*128]  fp16   partition-major Wr-hi (+ br-hi row)
      d16s [128, KT*128]  fp16   partition-major (Wr - c)*2048 (+ br-lo row)
      c8   [128, KT2*256] fp8    partition-major fp8 copy of c (fp8-padded)
      mats16 [128, NM16*128] bf16
      mats32 [128, NM32*128] fp32
      biasp  [128, NBIAS] fp32
    """
    f32 = np.float32
    Wr = np.asarray(Wr, f32)
    br = np.asarray(br, f32)
    We = np.asarray(We, f32)
    be = np.asarray(be, f32)
    Wq = np.asarray(Wq, f32)
    bq = np.asarray(bq, f32)
    Wk = np.asarray(Wk, f32)
    bk = np.asarray(bk, f32)
    Wv = np.asarray(Wv, f32)
    bv = np.asarray(bv, f32)
    Wo = np.asarray(Wo, f32)
    bo = np.asarray(bo, f32)

    # router weight split; bias br rides on the ones-row (index TD)
    w_full = np.zeros((TDP8, GE), f32)
    w_full[:TD] = Wr
    w_full[TD] = br
    c_full = w_full.astype(np.float16).astype(f32)
    d_full = ((w_full - c_full) * RS).astype(np.float16).astype(f32)

    def pmajor(a, kt):  # [kt*128, GE] -> [128, kt*128] partition-major
        return np.ascontiguousarray(
            a[:kt * 128].reshape(kt, 128, GE).transpose(1, 0, 2).reshape(128, kt * GE))

    c16 = pmajor(c_full, KT).astype(np.float16)
    d16s = pmajor(d_full, KT).astype(np.float16)
    c8 = pmajor(c_full, KT2 * 2).astype(ml_dtypes.float8_e4m3fn)

    mats = np.zeros((NM16, GE, GE), f32)
    biasp = np.zeros((GE, NBIAS), f32)

    mats[M_WET] = np.transpose(We, (2, 0, 1)).reshape(GE, GE)
    biasp[:, B_BE] = be.reshape(GE)
    biasp[:, B_BO] = bo.reshape(GE)

    d_ = np.arange(DH)
    for g in range(G):
        for h in range(H):
            for d in range(DH):
                p = d * 32 + g * 4 + h
                mats[M_AQ, g * 16:(g + 1) * 16, p] = Wq[g, h * 4 + d, :]
                biasp[p, B_BQ] = bq[g, h * 4 + d]
            for e in range(DH):
                ps = d_ * 32 + g * 4 + h
                for p in ps:
                    mats[M_AK0 + e, g * 16:(g + 1) * 16, p] = Wk[g, h * 4 + e, :]
                    mats[M_AV0 + e, g * 16:(g + 1) * 16, p] = Wv[g, h * 4 + e, :]
                    biasp[p, B_BK0 + e] = bk[g, h * 4 + e]
                    biasp[p, B_BV0 + e] = bv[g, h * 4 + e]
    for e in range(DH):
        for d in range(DH):
            for g in range(G):
                for h in range(H):
                    mats[M_MS0 + e, d * 32 + g * 4 + h, e * 32 + d * 8 + g] = 1.0
                    mats[M_MDEN, e * 32 + d * 8 + g, d * 32 + g * 4 + h] = 1.0
                    mats[M_MER0 + e, e * 32 + d * 8 + g, d * 32 + g * 4 + h] = 1.0
    for g in range(G):
        for f in range(E):
            for h in range(H):
                for d in range(DH):
                    mats[M_AO, d * 32 + g * 4 + h, g * 16 + f] = Wo[g, f, h * 4 + d]
    mats[M_IDT] = np.eye(GE, dtype=f32)
    mats[M_ONES] = 1.0

    mats16 = np.ascontiguousarray(
        np.transpose(mats, (1, 0, 2)).reshape(GE, NM16 * GE)).astype(ml_dtypes.bfloat16)

    m32 = np.zeros((NM32, GE, GE), f32)
    m32[M32_IDT] = np.eye(GE, dtype=f32)
    m32[M32_ONES] = 1.0
    mats32 = np.ascontiguousarray(np.transpose(m32, (1, 0, 2)).reshape(GE, NM32 * GE))
    return c16, d16s, c8, mats16, mats32, biasp


def prep_x_shard(xs):
    """xs [NSH, TD] fp32 -> (a16 [TDP, NSH] fp16, b8 [KT2,128,NCH,2*CH] fp8).

    a16 row TD is all-ones (carries the router bias); b8 is the scaled
    residual (x - fp16(x)) * 2048, pair-of-K-chunks packed and column
    pre-blocked per compute chunk so DMA runs are 1000 B.
    """
    f32 = np.float32
    xt = np.zeros((TDP8, NSH), f32)
    xt[:TD] = xs.T
    xt[TD] = 1.0                            # ones-row carries the router bias
    a = xt[:TDP].astype(np.float16)         # row TD: fp16(1.0) exact
    b = xt * RS
    b[:TDP] = (xt[:TDP] - a.astype(f32)) * RS   # rows TD.. are 0
    b8 = b.astype(ml_dtypes.float8_e4m3fn)  # [TDP8, NSH]
    # [KT2, 2, 128, NCH, CH] -> [KT2, 128, NCH, 2, CH] -> flatten last two
    b8 = b8.reshape(KT2, 2, 128, NCH, CH).transpose(0, 2, 3, 1, 4)
    b8 = np.ascontiguousarray(b8.reshape(KT2, 128, NCH, 2 * CH))
    return np.ascontiguousarray(a), b8


def build_kernel():
    """Trace the Bass/Tile kernel; returns the compiled Bacc."""
    nc = bacc.Bacc("TRN2", target_bir_lowering=False, debug=False,
                   num_devices=NCORES)

    a_d = nc.dram_tensor("a16", [TDP, NSH], F16, kind="ExternalInput").ap()
    b_d = nc.dram_tensor("b8", [KT2, 128, NCH, 2 * CH], F8, kind="ExternalInput").ap()
    c16_d = nc.dram_tensor("c16", [128, KT * 128], F16, kind="ExternalInput").ap()
    d16_d = nc.dram_tensor("d16s", [128, KT * 128], F16, kind="ExternalInput").ap()
    c8_d = nc.dram_tensor("c8", [128, KT2 * 256], F8, kind="ExternalInput").ap()
    m16_d = nc.dram_tensor("mats16", [128, NM16 * 128], BF16, kind="ExternalInput").ap()
    m32_d = nc.dram_tensor("mats32", [128, NM32 * 128], F32, kind="ExternalInput").ap()
    bias_d = nc.dram_tensor("bias", [128, NBIAS], F32, kind="ExternalInput").ap()
    out_d = nc.dram_tensor("out", [1, NSH], F32, kind="ExternalOutput").ap()

    AFT = mybir.ActivationFunctionType

    with tile.TileContext(nc) as tc:
        with (
            tc.tile_pool(name="consts", bufs=1) as consts,
            tc.tile_pool(name="xa", bufs=4) as xa,
            tc.tile_pool(name="xb", bufs=2) as xb,
            tc.tile_pool(name="work", bufs=1) as work,
            tc.tile_pool(name="rt", bufs=4, space="PSUM") as rtp,
            tc.tile_pool(name="pt", bufs=4, space="PSUM") as ptp,
        ):
            # ---- constants ----
            c16_sb = consts.tile([128, KT, 128], F16, tag="c16")
            nc.sync.dma_start(out=c16_sb, in_=c16_d.rearrange("p (t m) -> p t m", t=KT))
            d16_sb = consts.tile([128, KT, 128], F16, tag="d16")
            nc.sync.dma_start(out=d16_sb, in_=d16_d.rearrange("p (t m) -> p t m", t=KT))
            c8_sb = consts.tile([128, KT2 * 2, 128], F8, tag="c8")
            nc.sync.dma_start(out=c8_sb, in_=c8_d.rearrange("p (t m) -> p t m", t=KT2 * 2))
            m16_sb = consts.tile([128, NM16 * 128], BF16, tag="m16")
            nc.sync.dma_start(out=m16_sb, in_=m16_d)
            m32_sb = consts.tile([128, NM32 * 128], F32, tag="m32")
            nc.sync.dma_start(out=m32_sb, in_=m32_d)
            bias_sb = consts.tile([128, NBIAS], F32, tag="bias")
            nc.sync.dma_start(out=bias_sb, in_=bias_d)

            def mat16(i):
                return m16_sb[:, i * 128:(i + 1) * 128]

            def mat32(i):
                return m32_sb[:, i * 128:(i + 1) * 128]

            def bcol(i):
                return bias_sb[:, i:i + 1]

            idt32 = mat32(M32_IDT)
            ones32 = mat32(M32_ONES)
            idt16 = mat16(M_IDT)
            ones16 = mat16(M_ONES)

            def router(c):
                """Three-pass router accumulation for chunk c."""
                sl = slice(c * CH, (c + 1) * CH)
                main_ps = rtp.tile([128, CH], F32, tag="rt", name=f"main{c}")
                aux_ps = rtp.tile([128, CH], F32, tag="rt", name=f"aux{c}")
                for q in range(NQ):
                    at = xa.tile([128, KQ, CH], F16, tag="a", name=f"a{c}_{q}")
                    nc.sync.dma_start(
                        out=at,
                        in_=a_d[q * KQ * 128:(q + 1) * KQ * 128, sl].rearrange(
                            "(t p) j -> p t j", p=128))
                    for t in range(KQ):
                        tg = q * KQ + t
                        nc.tensor.matmul(main_ps[:, :], lhsT=c16_sb[:, tg, :],
                                         rhs=at[:, t, :],
                                         start=(tg == 0), stop=(tg == KT - 1))
                        nc.tensor.matmul(aux_ps[:, :], lhsT=d16_sb[:, tg, :],
                                         rhs=at[:, t, :],
                                         start=(tg == 0), stop=False)
                for hh in range(2):
                    t2n = KT2 // 2
                    bt = xb.tile([128, t2n, 2 * CH], F8, tag="b", name=f"b{c}_{hh}")
                    nc.sync.dma_start(
                        out=bt,
                        in_=b_d[hh * t2n:(hh + 1) * t2n, :, c, :].rearrange(
                            "t p m -> p t m"))
                    for t2 in range(t2n):
                        for s in range(2):
                            tt = (hh * t2n + t2) * 2 + s
                            nc.tensor.matmul(
                                aux_ps[:, :], lhsT=c8_sb[:, tt, :],
                                rhs=bt[:, t2, s * CH:(s + 1) * CH],
                                start=False, stop=(tt == KT2 * 2 - 1))
                return main_ps, aux_ps

            def post(c, main_ps, aux_ps):
                """Gating + experts + attention + combine for chunk c."""
                aux_sb = work.tile([128, CH], F32, tag="auxs")
                nc.scalar.activation(aux_sb, aux_ps[:, :], AFT.Identity,
                                     scale=1.0 / RS)
                h_sb = work.tile([128, CH], F32, tag="h")
                nc.vector.tensor_add(h_sb, main_ps[:, :], aux_sb)
                h16 = work.tile([128, CH], BF16, tag="h16")
                nc.scalar.activation(h16, h_sb, AFT.Copy)

                # --- exact top-2 threshold (second max per row) ---
                m2t_ps = ptp.tile([1, CH], F32, tag="pt", name=f"m2t{c}")
                for blk in range(4):
                    off = blk * 128
                    cs = min(128, CH - off)
                    tr = ptp.tile([128, 128], F32, tag="pt", name=f"tr{c}_{blk}")
                    nc.tensor.transpose(tr[:cs, :GE], h_sb[:, off:off + cs], idt32)
                    mx1 = work.tile([128, 1], F32, tag="mx1")
                    nc.vector.reduce_max(mx1[:cs], tr[:cs, :GE],
                                         axis=mybir.AxisListType.X)
                    eqm = work.tile([128, GE], F32, tag="eqm")
                    nc.vector.tensor_scalar(eqm[:cs], tr[:cs, :GE], mx1[:cs],
                                            None, op0=mybir.AluOpType.is_ge)
                    hm = work.tile([128, GE], F32, tag="hm")
                    nc.vector.scalar_tensor_tensor(
                        hm[:cs], in0=eqm[:cs], scalar=-1e30, in1=tr[:cs, :GE],
                        op0=mybir.AluOpType.mult, op1=mybir.AluOpType.add)
                    mx2 = work.tile([128, 1], F32, tag="mx2")
                    nc.vector.reduce_max(mx2[:cs], hm[:cs],
                                         axis=mybir.AxisListType.X)
                    # per-row threshold back into row-vector layout
                    nc.tensor.matmul(m2t_ps[0:1, off:off + cs], lhsT=mx2[:cs],
                                     rhs=idt32[:cs, :cs], start=True, stop=True)
                m2t_sb = work.tile([1, CH], F32, tag="m2t")
                nc.scalar.activation(m2t_sb, m2t_ps[:, :], AFT.Copy)
                # exact fp32 broadcast down 128 partitions (K=1 matmul)
                m2b_ps = ptp.tile([128, CH], F32, tag="pt", name=f"m2b{c}")
                nc.tensor.matmul(m2b_ps[:, :], lhsT=ones32[0:1, :],
                                 rhs=m2t_sb[0:1, :], start=True, stop=True)
                mask16 = work.tile([128, CH], BF16, tag="mask")
                nc.vector.tensor_tensor(mask16, h_sb, m2b_ps[:, :],
                                        op=mybir.AluOpType.is_ge)
                eh16 = work.tile([128, CH], BF16, tag="eh")
                nc.scalar.activation(eh16, h_sb, AFT.Exp)
                m1_16 = work.tile([128, CH], BF16, tag="m1")
                nc.vector.tensor_mul(m1_16, eh16, mask16)

                # --- experts + attention ---
                eo_ps = ptp.tile([128, CH], F32, tag="pt", name=f"eo{c}")
                nc.tensor.matmul(eo_ps[:, :], lhsT=mat16(M_WET), rhs=h16,
                                 start=True, stop=True)
                eo16 = work.tile([128, CH], BF16, tag="eo")
                nc.scalar.activation(eo16, eo_ps[:, :], AFT.Identity,
                                     bias=bcol(B_BE), scale=1.0)

                q_ps = ptp.tile([128, CH], F32, tag="pt", name=f"q{c}")
                nc.tensor.matmul(q_ps[:, :], lhsT=mat16(M_AQ), rhs=eo16,
                                 start=True, stop=True)
                qt16 = work.tile([128, CH], BF16, tag="qt")
                nc.scalar.activation(qt16, q_ps[:, :], AFT.Identity,
                                     bias=bcol(B_BQ), scale=1.0)

                sc_ps = ptp.tile([128, CH], F32, tag="pt", name=f"sc{c}")
                for e in range(DH):
                    kr_ps = ptp.tile([128, CH], F32, tag="pt", name=f"kr{c}_{e}")
                    nc.tensor.matmul(kr_ps[:, :], lhsT=mat16(M_AK0 + e), rhs=eo16,
                                     start=True, stop=True)
                    kr16 = work.tile([128, CH], BF16, tag="kr")
                    nc.scalar.activation(kr16, kr_ps[:, :], AFT.Identity,
                                         bias=bcol(B_BK0 + e), scale=1.0)
                    pe16 = work.tile([128, CH], BF16, tag="pe")
                    nc.vector.tensor_mul(pe16, qt16, kr16)
                    nc.tensor.matmul(sc_ps[:, :], lhsT=mat16(M_MS0 + e),
                                     rhs=pe16, start=(e == 0), stop=(e == DH - 1))
                es16 = work.tile([128, CH], BF16, tag="es")
                nc.scalar.activation(es16, sc_ps[:, :], AFT.Exp, scale=0.5)

                den_ps = ptp.tile([128, CH], F32, tag="pt", name=f"den{c}")
                nc.tensor.matmul(den_ps[:, :], lhsT=mat16(M_MDEN), rhs=es16,
                                 start=True, stop=True)
                drec = work.tile([128, CH], F32, tag="drec")
                nc.vector.reciprocal(drec, den_ps[:, :])

                att_ps = ptp.tile([128, CH], F32, tag="pt", name=f"att{c}")
                for e in range(DH):
                    vr_ps = ptp.tile([128, CH], F32, tag="pt", name=f"vr{c}_{e}")
                    nc.tensor.matmul(vr_ps[:, :], lhsT=mat16(M_AV0 + e), rhs=eo16,
                                     start=True, stop=True)
                    vr16 = work.tile([128, CH], BF16, tag="vr")
                    nc.scalar.activation(vr16, vr_ps[:, :], AFT.Identity,
                                         bias=bcol(B_BV0 + e), scale=1.0)
                    er_ps = ptp.tile([128, CH], F32, tag="pt", name=f"er{c}_{e}")
                    nc.tensor.matmul(er_ps[:, :], lhsT=mat16(M_MER0 + e), rhs=es16,
                                     start=True, stop=True)
                    pr16 = work.tile([128, CH], BF16, tag="pr")
                    nc.vector.tensor_mul(pr16, er_ps[:, :], vr16)
                    nc.tensor.matmul(att_ps[:, :], lhsT=idt16, rhs=pr16,
                                     start=(e == 0), stop=(e == DH - 1))
                att16 = work.tile([128, CH], BF16, tag="att")
                nc.vector.tensor_mul(att16, att_ps[:, :], drec)

                ao_ps = ptp.tile([128, CH], F32, tag="pt", name=f"ao{c}")
                nc.tensor.matmul(ao_ps[:, :], lhsT=mat16(M_AO), rhs=att16,
                                 start=True, stop=True)
                aout16 = work.tile([128, CH], BF16, tag="aout")
                nc.scalar.activation(aout16, ao_ps[:, :], AFT.Identity,
                                     bias=bcol(B_BO), scale=1.0)

                # --- weighted combine ---
                num16 = work.tile([128, CH], BF16, tag="num")
                nc.vector.tensor_mul(num16, m1_16, aout16)
                dens_ps = ptp.tile([1, CH], F32, tag="pt", name=f"dens{c}")
                nc.tensor.matmul(dens_ps[:, :], lhsT=ones16[:, 0:1], rhs=m1_16,
                                 start=True, stop=True)
                nums_ps = ptp.tile([1, CH], F32, tag="pt", name=f"nums{c}")
                nc.tensor.matmul(nums_ps[:, :], lhsT=ones16[:, 0:1], rhs=num16,
                                 start=True, stop=True)
                rden = work.tile([1, CH], F32, tag="rden")
                nc.vector.reciprocal(rden, dens_ps[:, :])
                pred = work.tile([1, CH], F32, tag="pred", bufs=2)
                nc.vector.tensor_mul(pred, nums_ps[:, :], rden)
                # out DMA from the ACT queue so SP keeps streaming x tiles
                nc.scalar.dma_start(out=out_d[0:1, c * CH:(c + 1) * CH], in_=pred)

            # pipeline: router(c+1) emitted before post(c)
            pending = router(0)
            for c in range(1, NCH):
                nxt = router(c)
                post(c - 1, *pending)
                pending = nxt
            post(NCH - 1, *pending)

    nc.compile()
    return nc


_NC_CACHE = None
LAST_RESULTS = None


def kernel(x, Wr, br, We, be, Wq, bq, Wk, bk, Wv, bv, Wo, bo):
    global _NC_CACHE, LAST_RESULTS
    f32 = np.float32
    x = np.asarray(x, f32)

    c16, d16s, c8, mats16, mats32, biasp = build_consts(
        Wr, br, We, be, Wq, bq, Wk, bk, Wv, bv, Wo, bo)

    if _NC_CACHE is None:
        _NC_CACHE = build_kernel()
    nc = _NC_CACHE

    in_maps = []
    for c in range(NCORES):
        xs = x[c * NSH:(c + 1) * NSH].reshape(NSH, TD)
        a16, b8 = prep_x_shard(xs)
        in_maps.append({"a16": a16, "b8": b8, "c16": c16, "d16s": d16s,
                        "c8": c8, "mats16": mats16, "mats32": mats32,
                        "bias": biasp})

    res = run_bass_kernel_spmd(nc, in_maps, list(range(NCORES)))
    LAST_RESULTS = res
    out = np.concatenate([res.results[c]["out"].reshape(NSH)
                          for c in range(NCORES)])
    return out.astype(f32)


# revision 14
# speedup vs baseline: 1.9679x; 1.3289x over previous
"""Trainium2 Bass kernel for nn_MIGAModel (moe_routing).

Pure data parallel over the stock axis N (8 cores, 2500 rows each).

Router precision scheme (the top-2 gating is discontinuous in the router
logits h, so h must be fp32-accurate to ~1e-5; plain fp16/bf16/fp32r
inputs all flip expert selections and fail the 2e-2 gate):
    x  = a + b      a = fp16(x),  b8 = fp8e4m3(b * 2048)
    Wr = c + d      c = fp16(Wr), d16 = fp16(d * 2048)
    h  = a@c  +  (a@d16 + b8@c8) / 2048        (c8 = fp8(c))
Two fp16 passes at 1 cyc/row plus one fp8 DoubleRow pass at 0.5 cyc/row
(pairs of K-chunks per instruction), two PSUM banks (main, aux), one
ACT + one DVE op to combine.  The router bias br rides in as an extra
contraction row (a row of ones in `a`, br split across c/d16), so
selection sees the exact biased logits.  delta-h ~1e-5 -> end-to-end
rel err ~3e-3 (selection flips dominate; measured in numpy and on HW).

DMA: a is 2 B/elem, b8 1 B/elem -> ~72 MB/core vs 95 MB for fp32.
Post-processing (experts + inner-group attention as 128x128
block-diagonal matmuls) runs on bf16 operands (1 cyc/row), biases are
folded into ACT-engine PSUM->SBUF moves.  The top-2 threshold test runs
in the PE-transposed space (rows on partitions) where the per-row
second-max is a [128,1] tensor_scalar operand, and the 0/1 mask is
transposed back by cheap bf16 PE transposes — no fp32 broadcast
matmuls.

Scheduling: the post chain of chunk c is a latency-bound
PE<->DVE<->ACT ping-pong, so its PE instructions are interleaved into
chunk c+1's router matmul stream in small groups — the tensor engine
never idles (which would also reset the cost model's p-state ramp).
Chunk widths shrink toward the end ([500,500,500,400,344,256]) so the
final, un-hidden post chain is as short as possible.  Output DMAs
issue from the ACT queue so the SP queue (x tiles) keeps streaming.
"""
import sys
import numpy as np

for _p in ("/opt/trn_rl_repo",):
    if _p not in sys.path:
        sys.path.insert(0, _p)

import ml_dtypes

import concourse.bass as bass
import concourse.tile as tile
from concourse import bacc, mybir
from concourse.bass_utils import run_bass_kernel_spmd

F32 = mybir.dt.float32
F16 = mybir.dt.float16
F8 = mybir.dt.float8e4
BF16 = mybir.dt.bfloat16

N, T, D = 20000, 60, 158
TD = T * D                      # 9480
G, E, H, DH, GE = 8, 16, 4, 4, 128
NCORES = 8
NSH = N // NCORES               # 2500 rows per core
KT = 75                         # fp16 K-chunks of 128 (9600 padded, row 9480 = ones)
TDP = KT * 128                  # 9600
KT2 = 38                        # fp8 K-pairs (9728 padded)
NQ = 5                          # a-tile K-groups per chunk (15 K-chunks each)
KQ = KT // NQ                   # 15
RS = 2048.0                     # residual scale (2**11)

# compute chunk widths: tapered so the last (un-hidden) post chain is short;
# every width is >=256 so fp16 a-tile DMA runs stay >=512 B.
WIDTHS = [500, 500, 500, 400, 344, 256]
NCH = len(WIDTHS)
LOS = [sum(WIDTHS[:i]) for i in range(NCH)]
# fp8 per-sub-row padded widths (pair stride must be a multiple of 16 B)
W8S = [(w + 15) // 16 * 16 for w in WIDTHS]
B8OFF = [2 * sum(W8S[:i]) for i in range(NCH)]
B8TOT = 2 * sum(W8S)

USE_DOUBLE_ROW = True

# bf16 packed matrix indices ([128,128] blocks in "mats16")
M_WET, M_AQ = 0, 1
M_AK0, M_AV0 = 2, 6             # 4 each
M_MS0 = 10                      # 4
M_MDEN = 14
M_MER0 = 15                     # 4
M_AO = 19
M_IDT = 20
M_ONES = 21
NM16 = 22

# fp32 packed matrices: identity (fp32 transposes)
M32_IDT = 0
NM32 = 1

# bias pack columns (fp32)
B_BE, B_BQ, B_BK0, B_BV0, B_BO = 0, 1, 2, 6, 10
NBIAS = 11


def build_consts(Wr, br, We, be, Wq, bq, Wk, bk, Wv, bv, Wo, bo):
    """Host-side packed constants (see build_kernel for layouts)."""
    f32 = np.float32
    Wr = np.asarray(Wr, f32)
    br = np.asarray(br, f32)
    We = np.asarray(We, f32)
    be = np.asarray(be, f32)
    Wq = np.asarray(Wq, f32)
    bq = np.asarray(bq, f32)
    Wk = np.asarray(Wk, f32)
    bk = np.asarray(bk, f32)
    Wv = np.asarray(Wv, f32)
    bv = np.asarray(bv, f32)
    Wo = np.asarray(Wo, f32)
    bo = np.asarray(bo, f32)

    # router weight split; bias br rides on the ones-row (index TD)
    w_full = np.zeros((KT2 * 256, GE), f32)
    w_full[:TD] = Wr
    w_full[TD] = br
    c_full = w_full.astype(np.float16).astype(f32)
    d_full = ((w_full - c_full) * RS).astype(np.float16).astype(f32)

    def pmajor(a, kt):  # [kt*128, GE] -> [128, kt*128] partition-major
        return np.ascontiguousarray(
            a[:kt * 128].reshape(kt, 128, GE).transpose(1, 0, 2).reshape(128, kt * GE))

    c16 = pmajor(c_full, KT).astype(np.float16)
    d16s = pmajor(d_full, KT).astype(np.float16)
    c8 = pmajor(c_full, KT2 * 2).astype(ml_dtypes.float8_e4m3fn)

    mats = np.zeros((NM16, GE, GE), f32)
    biasp = np.zeros((GE, NBIAS), f32)

    mats[M_WET] = np.transpose(We, (2, 0, 1)).reshape(GE, GE)
    biasp[:, B_BE] = be.reshape(GE)
    biasp[:, B_BO] = bo.reshape(GE)

    d_ = np.arange(DH)
    for g in range(G):
        for h in range(H):
            for d in range(DH):
                p = d * 32 + g * 4 + h
                mats[M_AQ, g * 16:(g + 1) * 16, p] = Wq[g, h * 4 + d, :]
                biasp[p, B_BQ] = bq[g, h * 4 + d]
            for e in range(DH):
                ps = d_ * 32 + g * 4 + h
                for p in ps:
                    mats[M_AK0 + e, g * 16:(g + 1) * 16, p] = Wk[g, h * 4 + e, :]
                    mats[M_AV0 + e, g * 16:(g + 1) * 16, p] = Wv[g, h * 4 + e, :]
                    biasp[p, B_BK0 + e] = bk[g, h * 4 + e]
                    biasp[p, B_BV0 + e] = bv[g, h * 4 + e]
    for e in range(DH):
        for d in range(DH):
            for g in range(G):
                for h in range(H):
                    mats[M_MS0 + e, d * 32 + g * 4 + h, e * 32 + d * 8 + g] = 1.0
                    mats[M_MDEN, e * 32 + d * 8 + g, d * 32 + g * 4 + h] = 1.0
                    mats[M_MER0 + e, e * 32 + d * 8 + g, d * 32 + g * 4 + h] = 1.0
    for g in range(G):
        for f in range(E):
            for h in range(H):
                for d in range(DH):
                    mats[M_AO, d * 32 + g * 4 + h, g * 16 + f] = Wo[g, f, h * 4 + d]
    mats[M_IDT] = np.eye(GE, dtype=f32)
    mats[M_ONES] = 1.0

    mats16 = np.ascontiguousarray(
        np.transpose(mats, (1, 0, 2)).reshape(GE, NM16 * GE)).astype(ml_dtypes.bfloat16)

    m32 = np.zeros((NM32, GE, GE), f32)
    m32[M32_IDT] = np.eye(GE, dtype=f32)
    mats32 = np.ascontiguousarray(np.transpose(m32, (1, 0, 2)).reshape(GE, NM32 * GE))
    return c16, d16s, c8, mats16, mats32, biasp


def prep_x_shard(xs):
    """xs [NSH, TD] fp32 -> (a16 [TDP, NSH] fp16, b8 [KT2, 128, B8TOT] fp8).

    a16 row TD is all-ones (carries the router bias); b8 is the scaled
    residual (x - fp16(x)) * 2048, pair-of-K-chunks packed and column
    pre-blocked per compute chunk (chunk widths padded per sub-row so
    the DoubleRow pair stride is a multiple of 16 B and DMA runs are
    >=512 B).
    """
    f32 = np.float32
    xt = np.zeros((KT2 * 256, NSH), f32)
    xt[:TD] = xs.T
    xt[TD] = 1.0                            # ones-row carries the router bias
    a = xt[:TDP].astype(np.float16)         # row TD: fp16(1.0) exact
    b = xt * RS
    b[:TDP] = (xt[:TDP] - a.astype(f32)) * RS   # rows TD.. stay 0
    b8s = np.asarray(b.astype(ml_dtypes.float8_e4m3fn))  # [KT2*256, NSH]
    b8s = b8s.reshape(KT2, 2, 128, NSH)
    b8 = np.zeros((KT2, 128, B8TOT), ml_dtypes.float8_e4m3fn)
    for c in range(NCH):
        lo, w, w8, off = LOS[c], WIDTHS[c], W8S[c], B8OFF[c]
        for s in range(2):
            b8[:, :, off + s * w8: off + s * w8 + w] = b8s[:, s, :, lo:lo + w]
    return np.ascontiguousarray(a), b8


def build_kernel():
    """Trace the Bass/Tile kernel; returns the compiled Bacc."""
    nc = bacc.Bacc("TRN2", target_bir_lowering=False, debug=False,
                   num_devices=NCORES)

    a_d = nc.dram_tensor("a16", [TDP, NSH], F16, kind="ExternalInput").ap()
    b_d = nc.dram_tensor("b8", [KT2, 128, B8TOT], F8, kind="ExternalInput").ap()
    c16_d = nc.dram_tensor("c16", [128, KT * 128], F16, kind="ExternalInput").ap()
    d16_d = nc.dram_tensor("d16s", [128, KT * 128], F16, kind="ExternalInput").ap()
    c8_d = nc.dram_tensor("c8", [128, KT2 * 256], F8, kind="ExternalInput").ap()
    m16_d = nc.dram_tensor("mats16", [128, NM16 * 128], BF16, kind="ExternalInput").ap()
    m32_d = nc.dram_tensor("mats32", [128, NM32 * 128], F32, kind="ExternalInput").ap()
    bias_d = nc.dram_tensor("bias", [128, NBIAS], F32, kind="ExternalInput").ap()
    out_d = nc.dram_tensor("out", [1, NSH], F32, kind="ExternalOutput").ap()

    AFT = mybir.ActivationFunctionType

    with tile.TileContext(nc) as tc:
        with (
            tc.tile_pool(name="consts", bufs=1) as consts,
            tc.tile_pool(name="xa", bufs=3) as xa,
            tc.tile_pool(name="xb", bufs=2) as xb,
            tc.tile_pool(name="work", bufs=1) as work,
            tc.tile_pool(name="rt", bufs=4, space="PSUM") as rtp,
            tc.tile_pool(name="pt", bufs=4, space="PSUM") as ptp,
        ):
            # ---- constant tiles (DMAs issued inside the chunk-0 stream) ----
            c16_sb = consts.tile([128, KT, 128], F16, tag="c16")
            d16_sb = consts.tile([128, KT, 128], F16, tag="d16")
            c8_sb = consts.tile([128, KT2 * 2, 128], F8, tag="c8")
            m16_sb = consts.tile([128, NM16 * 128], BF16, tag="m16")
            m32_sb = consts.tile([128, NM32 * 128], F32, tag="m32")
            bias_sb = consts.tile([128, NBIAS], F32, tag="bias")

            def mat16(i):
                return m16_sb[:, i * 128:(i + 1) * 128]

            def bcol(i):
                return bias_sb[:, i:i + 1]

            idt32 = m32_sb[:, M32_IDT * 128:(M32_IDT + 1) * 128]
            idt16 = mat16(M_IDT)
            ones16 = mat16(M_ONES)

            def dma_cd_piece(k):
                """k-th quarter of the c16/d16s constants (19 K-chunks)."""
                t0, t1 = k * 19, min(KT, (k + 1) * 19)
                nc.sync.dma_start(
                    out=c16_sb[:, t0:t1, :],
                    in_=c16_d[:, t0 * 128:t1 * 128].rearrange(
                        "p (t m) -> p t m", m=128))
                nc.sync.dma_start(
                    out=d16_sb[:, t0:t1, :],
                    in_=d16_d[:, t0 * 128:t1 * 128].rearrange(
                        "p (t m) -> p t m", m=128))

            def router_thunks(c):
                """DMA + matmul thunk list for chunk c's router passes."""
                lo, w, w8, boff = LOS[c], WIDTHS[c], W8S[c], B8OFF[c]
                sl = slice(lo, lo + w)
                main_ps = rtp.tile([128, w], F32, tag="rt", name=f"main{c}")
                aux_ps = rtp.tile([128, w], F32, tag="rt", name=f"aux{c}")
                thunks = []
                atiles = [None] * NQ
                btiles = [None, None]

                def dma_a(q):
                    at = xa.tile([128, KQ, w], F16, tag="a", name=f"a{c}_{q}")
                    nc.sync.dma_start(
                        out=at,
                        in_=a_d[q * KQ * 128:(q + 1) * KQ * 128, sl].rearrange(
                            "(t p) j -> p t j", p=128))
                    atiles[q] = at

                def dma_b(hh):
                    t2n = KT2 // 2
                    bt = xb.tile([128, t2n, 2 * w8], F8, tag="b", name=f"b{c}_{hh}")
                    nc.sync.dma_start(
                        out=bt,
                        in_=b_d[hh * t2n:(hh + 1) * t2n, :,
                                boff:boff + 2 * w8].rearrange("t p m -> p t m"))
                    btiles[hh] = bt

                for q in range(NQ):
                    def pre(q=q):
                        dma_a(q)
                        if c == 0:
                            if q < 4:
                                dma_cd_piece(q)
                            if q == 3:
                                nc.sync.dma_start(
                                    out=c8_sb,
                                    in_=c8_d.rearrange("p (t m) -> p t m", m=128))
                                dma_b(0)
                            elif q == 4:
                                dma_b(1)
                                nc.sync.dma_start(out=m16_sb, in_=m16_d)
                                nc.sync.dma_start(out=m32_sb, in_=m32_d)
                                nc.sync.dma_start(out=bias_sb, in_=bias_d)
                        else:
                            if q == 3:
                                dma_b(0)
                            elif q == 4:
                                dma_b(1)

                    for t in range(KQ):
                        def mm_main(q=q, t=t, pre=(pre if t == 0 else None)):
                            if pre:
                                pre()
                            tg = q * KQ + t
                            nc.tensor.matmul(main_ps[:, :], lhsT=c16_sb[:, tg, :],
                                             rhs=atiles[q][:, t, :],
                                             start=(tg == 0), stop=(tg == KT - 1))
                        thunks.append(mm_main)
                    for t in range(KQ):
                        def mm_aux(q=q, t=t):
                            tg = q * KQ + t
                            nc.tensor.matmul(aux_ps[:, :], lhsT=d16_sb[:, tg, :],
                                             rhs=atiles[q][:, t, :],
                                             start=(tg == 0), stop=False)
                        thunks.append(mm_aux)

                if USE_DOUBLE_ROW:
                    for g in range(KT2):
                        def mm_b(g=g):
                            hh, t2 = divmod(g, KT2 // 2)
                            rhs = btiles[hh][:, t2, :].rearrange(
                                "p (s j) -> p s j", s=2)[:, :, 0:w]
                            nc.tensor.matmul(
                                aux_ps[:, :], lhsT=c8_sb[:, 2 * g:2 * g + 2, :],
                                rhs=rhs, start=False, stop=(g == KT2 - 1),
                                perf_mode=mybir.MatmulPerfMode.DoubleRow)
                        thunks.append(mm_b)
                else:
                    for g in range(KT2):
                        for s in range(2):
                            def mm_b(g=g, s=s):
                                hh, t2 = divmod(g, KT2 // 2)
                                nc.tensor.matmul(
                                    aux_ps[:, :], lhsT=c8_sb[:, 2 * g + s, :],
                                    rhs=btiles[hh][:, t2, s * w8:s * w8 + w],
                                    start=False,
                                    stop=(g == KT2 - 1 and s == 1))
                            thunks.append(mm_b)
                return thunks, main_ps, aux_ps

            def post_groups(c, main_ps, aux_ps):
                """Post chain for chunk c as (frac, thunk) groups.

                frac positions the group inside chunk c+1's router matmul
                stream; PE members' dependencies are produced well before
                the PE reaches them, so the tensor engine never stalls.
                """
                lo, w = LOS[c], WIDTHS[c]
                blks = [(off, min(128, w - off)) for off in range(0, w, 128)]
                st = {}

                def g_h():
                    aux_sb = work.tile([128, w], F32, tag="auxs", name="auxs")
                    nc.scalar.activation(aux_sb, aux_ps[:, :], AFT.Identity,
                                         scale=1.0 / RS)
                    st["h"] = work.tile([128, w], F32, tag="h", name="h")
                    nc.vector.tensor_add(st["h"], main_ps[:, :], aux_sb)
                    st["h16"] = work.tile([128, w], BF16, tag="h16", name="h16")
                    nc.scalar.activation(st["h16"], st["h"], AFT.Copy)

                def g_top2():
                    # per-row top-2 threshold + mask, in transposed space
                    st["trs"] = []
                    for blk, (off, cs) in enumerate(blks):
                        tr = ptp.tile([128, 128], F32, tag="pt", name=f"tr{c}_{blk}")
                        nc.tensor.transpose(tr[:cs, :GE], st["h"][:, off:off + cs],
                                            idt32)
                        mx1 = work.tile([128, 1], F32, tag="mx1", name="mx1")
                        nc.vector.reduce_max(mx1[:cs], tr[:cs, :GE],
                                             axis=mybir.AxisListType.X)
                        eqm = work.tile([128, GE], F32, tag="eqm", name="eqm")
                        nc.vector.tensor_scalar(eqm[:cs], tr[:cs, :GE], mx1[:cs],
                                                None, op0=mybir.AluOpType.is_ge)
                        hm = work.tile([128, GE], F32, tag="hm", name="hm")
                        nc.vector.scalar_tensor_tensor(
                            hm[:cs], in0=eqm[:cs], scalar=-1e30, in1=tr[:cs, :GE],
                            op0=mybir.AluOpType.mult, op1=mybir.AluOpType.add)
                        mx2 = work.tile([128, 1], F32, tag="mx2", name="mx2")
                        nc.vector.reduce_max(mx2[:cs], hm[:cs],
                                             axis=mybir.AxisListType.X)
                        mtr = work.tile([128, GE], BF16, tag="mtr", name="mtr")
                        nc.vector.tensor_scalar(mtr[:cs], tr[:cs, :GE], mx2[:cs],
                                                None, op0=mybir.AluOpType.is_ge)
                        st["trs"].append((mtr, off, cs))

                def g_maskback():
                    st["mask_ps"] = ptp.tile([128, w], BF16, tag="pt",
                                             name=f"maskps{c}")
                    for mtr, off, cs in st["trs"]:
                        nc.tensor.transpose(st["mask_ps"][:GE, off:off + cs],
                                            mtr[:cs, :GE], idt16[:cs, :cs])

                def g_gate():
                    eh16 = work.tile([128, w], BF16, tag="eh", name="eh")
                    nc.scalar.activation(eh16, st["h"], AFT.Exp)
                    st["m1"] = work.tile([128, w], BF16, tag="m1", name="m1")
                    nc.vector.tensor_mul(st["m1"], eh16, st["mask_ps"][:, :])

                def g_eo():
                    eo_ps = ptp.tile([128, w], F32, tag="pt", name=f"eo{c}")
                    nc.tensor.matmul(eo_ps[:, :], lhsT=mat16(M_WET), rhs=st["h16"],
                                     start=True, stop=True)
                    st["eo16"] = work.tile([128, w], BF16, tag="eo", name="eo")
                    nc.scalar.activation(st["eo16"], eo_ps[:, :], AFT.Identity,
                                         bias=bcol(B_BE), scale=1.0)

                def g_q():
                    q_ps = ptp.tile([128, w], F32, tag="pt", name=f"q{c}")
                    nc.tensor.matmul(q_ps[:, :], lhsT=mat16(M_AQ), rhs=st["eo16"],
                                     start=True, stop=True)
                    st["qt16"] = work.tile([128, w], BF16, tag="qt", name="qt")
                    nc.scalar.activation(st["qt16"], q_ps[:, :], AFT.Identity,
                                         bias=bcol(B_BQ), scale=1.0)
                    st["sc_ps"] = ptp.tile([128, w], F32, tag="pt", name=f"sc{c}")

                def g_kr(e):
                    kr_ps = ptp.tile([128, w], F32, tag="pt", name=f"kr{c}_{e}")
                    nc.tensor.matmul(kr_ps[:, :], lhsT=mat16(M_AK0 + e),
                                     rhs=st["eo16"], start=True, stop=True)
                    kr16 = work.tile([128, w], BF16, tag="kr", name="kr")
                    nc.scalar.activation(kr16, kr_ps[:, :], AFT.Identity,
                                         bias=bcol(B_BK0 + e), scale=1.0)
                    pe16 = work.tile([128, w], BF16, tag=f"pe{e}", name=f"pe{e}")
                    nc.vector.tensor_mul(pe16, st["qt16"], kr16)
                    st[f"pe{e}"] = pe16

                def g_ms(e):
                    nc.tensor.matmul(st["sc_ps"][:, :], lhsT=mat16(M_MS0 + e),
                                     rhs=st[f"pe{e}"], start=(e == 0),
                                     stop=(e == DH - 1))
                    if e == DH - 1:
                        st["es16"] = work.tile([128, w], BF16, tag="es", name="es")
                        nc.scalar.activation(st["es16"], st["sc_ps"][:, :],
                                             AFT.Exp, scale=0.5)

                def g_den():
                    den_ps = ptp.tile([128, w], F32, tag="pt", name=f"den{c}")
                    nc.tensor.matmul(den_ps[:, :], lhsT=mat16(M_MDEN),
                                     rhs=st["es16"], start=True, stop=True)
                    st["drec"] = work.tile([128, w], F32, tag="drec", name="drec")
                    nc.vector.reciprocal(st["drec"], den_ps[:, :])

                def g_vr(e):
                    vr_ps = ptp.tile([128, w], F32, tag="pt", name=f"vr{c}_{e}")
                    nc.tensor.matmul(vr_ps[:, :], lhsT=mat16(M_AV0 + e),
                                     rhs=st["eo16"], start=True, stop=True)
                    vr16 = work.tile([128, w], BF16, tag="vr", name="vr")
                    nc.scalar.activation(vr16, vr_ps[:, :], AFT.Identity,
                                         bias=bcol(B_BV0 + e), scale=1.0)
                    er_ps = ptp.tile([128, w], F32, tag="pt", name=f"er{c}_{e}")
                    nc.tensor.matmul(er_ps[:, :], lhsT=mat16(M_MER0 + e),
                                     rhs=st["es16"], start=True, stop=True)
                    pr16 = work.tile([128, w], BF16, tag=f"pr{e}", name=f"pr{e}")
                    nc.vector.tensor_mul(pr16, er_ps[:, :], vr16)
                    st[f"pr{e}"] = pr16

                def g_att():
                    s1 = work.tile([128, w], BF16, tag="s1", name="s1")
                    nc.vector.tensor_add(s1, st["pr0"], st["pr1"])
                    s2 = work.tile([128, w], BF16, tag="s2", name="s2")
                    nc.vector.tensor_add(s2, st["pr2"], st["pr3"])
                    s3 = work.tile([128, w], BF16, tag="s3", name="s3")
                    nc.vector.tensor_add(s3, s1, s2)
                    st["att16"] = work.tile([128, w], BF16, tag="att", name="att")
                    nc.vector.tensor_mul(st["att16"], s3, st["drec"])

                def g_ao():
                    ao_ps = ptp.tile([128, w], F32, tag="pt", name=f"ao{c}")
                    nc.tensor.matmul(ao_ps[:, :], lhsT=mat16(M_AO),
                                     rhs=st["att16"], start=True, stop=True)
                    aout16 = work.tile([128, w], BF16, tag="aout", name="aout")
                    nc.scalar.activation(aout16, ao_ps[:, :], AFT.Identity,
                                         bias=bcol(B_BO), scale=1.0)
                    st["num16"] = work.tile([128, w], BF16, tag="num", name="num")
                    nc.vector.tensor_mul(st["num16"], st["m1"], aout16)

                def g_fin():
                    dens_ps = ptp.tile([1, w], F32, tag="pt", name=f"dens{c}")
                    nc.tensor.matmul(dens_ps[:, :], lhsT=ones16[:, 0:1],
                                     rhs=st["m1"], start=True, stop=True)
                    nums_ps = ptp.tile([1, w], F32, tag="pt", name=f"nums{c}")
                    nc.tensor.matmul(nums_ps[:, :], lhsT=ones16[:, 0:1],
                                     rhs=st["num16"], start=True, stop=True)
                    rden = work.tile([1, w], F32, tag="rden", name="rden")
                    nc.vector.reciprocal(rden, dens_ps[:, :])
                    pred = work.tile([1, w], F32, tag="pred", name="pred", bufs=2)
                    nc.vector.tensor_mul(pred, nums_ps[:, :], rden)
                    nc.scalar.dma_start(out=out_d[0:1, lo:lo + w], in_=pred)

                return [
                    (0.00, g_h),
                    (0.06, g_top2),
                    (0.17, g_maskback),
                    (0.21, g_gate),
                    (0.25, g_eo),
                    (0.31, g_q),
                    (0.35, lambda: g_kr(0)),
                    (0.40, lambda: (g_ms(0), g_kr(1))),
                    (0.45, lambda: (g_ms(1), g_kr(2))),
                    (0.50, lambda: (g_ms(2), g_kr(3))),
                    (0.55, lambda: g_ms(3)),
                    (0.61, lambda: (g_den(), g_vr(0))),
                    (0.66, lambda: g_vr(1)),
                    (0.71, lambda: g_vr(2)),
                    (0.76, lambda: g_vr(3)),
                    (0.80, g_att),
                    (0.85, g_ao),
                    (0.92, g_fin),
                ]

            pending = None
            for c in range(NCH):
                thunks, main_ps, aux_ps = router_thunks(c)
                nmm = len(thunks)
                sched = {}
                if pending is not None:
                    for frac, fn in pending:
                        sched.setdefault(min(nmm - 1, int(frac * nmm)), []).append(fn)
                for i, t in enumerate(thunks):
                    t()
                    for fn in sched.get(i, ()):
                        fn()
                pending = post_groups(c, main_ps, aux_ps)
            for frac, fn in pending:
                fn()

    nc.compile()
    return nc


_NC_CACHE = None
LAST_RESULTS = None


def kernel(x, Wr, br, We, be, Wq, bq, Wk, bk, Wv, bv, Wo, bo):
    global _NC_CACHE, LAST_RESULTS
    f32 = np.float32
    x = np.asarray(x, f32)

    c16, d16s, c8, mats16, mats32, biasp = build_consts(
        Wr, br, We, be, Wq, bq, Wk, bk, Wv, bv, Wo, bo)

    if _NC_CACHE is None:
        _NC_CACHE = build_kernel()
    nc = _NC_CACHE

    in_maps = []
    for c in range(NCORES):
        xs = x[c * NSH:(c + 1) * NSH].reshape(NSH, TD)
        a16, b8 = prep_x_shard(xs)
        in_maps.append({"a16": a16, "b8": b8, "c16": c16, "d16s": d16s,
                        "c8": c8, "mats16": mats16, "mats32": mats32,
                        "bias": biasp})

    res = run_bass_kernel_spmd(nc, in_maps, list(range(NCORES)))
    LAST_RESULTS = res
    out = np.concatenate([res.results[c]["out"].reshape(NSH)
                          for c in range(NCORES)])
    return out.astype(f32)


# revision 15
# speedup vs baseline: 1.9841x; 1.0082x over previous
"""Trainium2 Bass kernel for nn_MIGAModel (moe_routing).

Pure data parallel over the stock axis N (8 cores, 2500 rows each).

Router precision scheme (the top-2 gating is discontinuous in the router
logits h, so h must be fp32-accurate to ~1e-5; plain fp16/bf16/fp32r
inputs all flip expert selections and fail the 2e-2 gate):
    x  = a + b      a = fp16(x),  b8 = fp8e4m3(b * 2048)
    Wr = c + d      c = fp16(Wr), d16 = fp16(d * 2048)
    h  = a@c  +  (a@d16 + b8@c8) / 2048        (c8 = fp8(c))
Two fp16 passes at 1 cyc/row plus one fp8 DoubleRow pass at 0.5 cyc/row
(pairs of K-chunks per instruction), two PSUM banks (main, aux), one
ACT + one DVE op to combine.  The router bias br rides in as an extra
contraction row (a row of ones in `a`, br split across c/d16), so
selection sees the exact biased logits.  delta-h ~1e-5 -> end-to-end
rel err ~3e-3 (selection flips dominate; measured in numpy and on HW).

DMA: a is 2 B/elem, b8 1 B/elem -> ~72 MB/core vs 95 MB for fp32.
Post-processing (experts + inner-group attention as 128x128
block-diagonal matmuls) runs on bf16 operands (1 cyc/row), biases are
folded into ACT-engine PSUM->SBUF moves.  The top-2 threshold test runs
in the PE-transposed space (rows on partitions) where the per-row
second-max is a [128,1] tensor_scalar operand, and the 0/1 mask is
transposed back by cheap bf16 PE transposes — no fp32 broadcast
matmuls.

Scheduling: the post chain of chunk c is a latency-bound
PE<->DVE<->ACT ping-pong, so its PE instructions are interleaved into
chunk c+1's router matmul stream in small groups — the tensor engine
never idles (which would also reset the cost model's p-state ramp).
Chunk widths taper at both ends ([344,500,500,500,400,256]) so the
first matmul starts early in the DMA stream and
final, un-hidden post chain is as short as possible.  Output DMAs
issue from the ACT queue so the SP queue (x tiles) keeps streaming.
"""
import sys
import numpy as np

for _p in ("/opt/trn_rl_repo",):
    if _p not in sys.path:
        sys.path.insert(0, _p)

import ml_dtypes

import concourse.bass as bass
import concourse.tile as tile
from concourse import bacc, mybir
from concourse.bass_utils import run_bass_kernel_spmd

F32 = mybir.dt.float32
F16 = mybir.dt.float16
F8 = mybir.dt.float8e4
BF16 = mybir.dt.bfloat16

N, T, D = 20000, 60, 158
TD = T * D                      # 9480
G, E, H, DH, GE = 8, 16, 4, 4, 128
NCORES = 8
NSH = N // NCORES               # 2500 rows per core
KT = 75                         # fp16 K-chunks of 128 (9600 padded, row 9480 = ones)
TDP = KT * 128                  # 9600
KT2 = 38                        # fp8 K-pairs (9728 padded)
NQ = 5                          # a-tile K-groups per chunk (15 K-chunks each)
KQ = KT // NQ                   # 15
RS = 2048.0                     # residual scale (2**11)

# compute chunk widths: tapered so the last (un-hidden) post chain is short;
# every width is >=256 so fp16 a-tile DMA runs stay >=512 B.
WIDTHS = [344, 500, 500, 500, 400, 256]
NCH = len(WIDTHS)
LOS = [sum(WIDTHS[:i]) for i in range(NCH)]
# fp8 per-sub-row padded widths (pair stride must be a multiple of 16 B)
W8S = [(w + 15) // 16 * 16 for w in WIDTHS]
B8OFF = [2 * sum(W8S[:i]) for i in range(NCH)]
B8TOT = 2 * sum(W8S)

USE_DOUBLE_ROW = True

# bf16 packed matrix indices ([128,128] blocks in "mats16")
M_WET, M_AQ = 0, 1
M_AK0, M_AV0 = 2, 6             # 4 each
M_MS0 = 10                      # 4
M_MDEN = 14
M_MER0 = 15                     # 4
M_AO = 19
M_IDT = 20
M_ONES = 21
NM16 = 22

# fp32 packed matrices: identity (fp32 transposes)
M32_IDT = 0
NM32 = 1

# bias pack columns (fp32)
B_BE, B_BQ, B_BK0, B_BV0, B_BO = 0, 1, 2, 6, 10
NBIAS = 11


def build_consts(Wr, br, We, be, Wq, bq, Wk, bk, Wv, bv, Wo, bo):
    """Host-side packed constants (see build_kernel for layouts)."""
    f32 = np.float32
    Wr = np.asarray(Wr, f32)
    br = np.asarray(br, f32)
    We = np.asarray(We, f32)
    be = np.asarray(be, f32)
    Wq = np.asarray(Wq, f32)
    bq = np.asarray(bq, f32)
    Wk = np.asarray(Wk, f32)
    bk = np.asarray(bk, f32)
    Wv = np.asarray(Wv, f32)
    bv = np.asarray(bv, f32)
    Wo = np.asarray(Wo, f32)
    bo = np.asarray(bo, f32)

    # router weight split; bias br rides on the ones-row (index TD)
    w_full = np.zeros((KT2 * 256, GE), f32)
    w_full[:TD] = Wr
    w_full[TD] = br
    c_full = w_full.astype(np.float16).astype(f32)
    d_full = ((w_full - c_full) * RS).astype(np.float16).astype(f32)

    def pmajor(a, kt):  # [kt*128, GE] -> [128, kt*128] partition-major
        return np.ascontiguousarray(
            a[:kt * 128].reshape(kt, 128, GE).transpose(1, 0, 2).reshape(128, kt * GE))

    c16 = pmajor(c_full, KT).astype(np.float16)
    d16s = pmajor(d_full, KT).astype(np.float16)
    c8 = pmajor(c_full, KT2 * 2).astype(ml_dtypes.float8_e4m3fn)

    mats = np.zeros((NM16, GE, GE), f32)
    biasp = np.zeros((GE, NBIAS), f32)

    mats[M_WET] = np.transpose(We, (2, 0, 1)).reshape(GE, GE)
    biasp[:, B_BE] = be.reshape(GE)
    biasp[:, B_BO] = bo.reshape(GE)

    d_ = np.arange(DH)
    for g in range(G):
        for h in range(H):
            for d in range(DH):
                p = d * 32 + g * 4 + h
                mats[M_AQ, g * 16:(g + 1) * 16, p] = Wq[g, h * 4 + d, :]
                biasp[p, B_BQ] = bq[g, h * 4 + d]
            for e in range(DH):
                ps = d_ * 32 + g * 4 + h
                for p in ps:
                    mats[M_AK0 + e, g * 16:(g + 1) * 16, p] = Wk[g, h * 4 + e, :]
                    mats[M_AV0 + e, g * 16:(g + 1) * 16, p] = Wv[g, h * 4 + e, :]
                    biasp[p, B_BK0 + e] = bk[g, h * 4 + e]
                    biasp[p, B_BV0 + e] = bv[g, h * 4 + e]
    for e in range(DH):
        for d in range(DH):
            for g in range(G):
                for h in range(H):
                    mats[M_MS0 + e, d * 32 + g * 4 + h, e * 32 + d * 8 + g] = 1.0
                    mats[M_MDEN, e * 32 + d * 8 + g, d * 32 + g * 4 + h] = 1.0
                    mats[M_MER0 + e, e * 32 + d * 8 + g, d * 32 + g * 4 + h] = 1.0
    for g in range(G):
        for f in range(E):
            for h in range(H):
                for d in range(DH):
                    mats[M_AO, d * 32 + g * 4 + h, g * 16 + f] = Wo[g, f, h * 4 + d]
    mats[M_IDT] = np.eye(GE, dtype=f32)
    mats[M_ONES] = 1.0

    mats16 = np.ascontiguousarray(
        np.transpose(mats, (1, 0, 2)).reshape(GE, NM16 * GE)).astype(ml_dtypes.bfloat16)

    m32 = np.zeros((NM32, GE, GE), f32)
    m32[M32_IDT] = np.eye(GE, dtype=f32)
    mats32 = np.ascontiguousarray(np.transpose(m32, (1, 0, 2)).reshape(GE, NM32 * GE))
    return c16, d16s, c8, mats16, mats32, biasp


def prep_x_shard(xs):
    """xs [NSH, TD] fp32 -> (a16 [TDP, NSH] fp16, b8 [KT2, 128, B8TOT] fp8).

    a16 row TD is all-ones (carries the router bias); b8 is the scaled
    residual (x - fp16(x)) * 2048, pair-of-K-chunks packed and column
    pre-blocked per compute chunk (chunk widths padded per sub-row so
    the DoubleRow pair stride is a multiple of 16 B and DMA runs are
    >=512 B).
    """
    f32 = np.float32
    xt = np.zeros((KT2 * 256, NSH), f32)
    xt[:TD] = xs.T
    xt[TD] = 1.0                            # ones-row carries the router bias
    a = xt[:TDP].astype(np.float16)         # row TD: fp16(1.0) exact
    b = xt * RS
    b[:TDP] = (xt[:TDP] - a.astype(f32)) * RS   # rows TD.. stay 0
    b8s = np.asarray(b.astype(ml_dtypes.float8_e4m3fn))  # [KT2*256, NSH]
    b8s = b8s.reshape(KT2, 2, 128, NSH)
    b8 = np.zeros((KT2, 128, B8TOT), ml_dtypes.float8_e4m3fn)
    for c in range(NCH):
        lo, w, w8, off = LOS[c], WIDTHS[c], W8S[c], B8OFF[c]
        for s in range(2):
            b8[:, :, off + s * w8: off + s * w8 + w] = b8s[:, s, :, lo:lo + w]
    return np.ascontiguousarray(a), b8


def build_kernel():
    """Trace the Bass/Tile kernel; returns the compiled Bacc."""
    nc = bacc.Bacc("TRN2", target_bir_lowering=False, debug=False,
                   num_devices=NCORES)

    a_d = nc.dram_tensor("a16", [TDP, NSH], F16, kind="ExternalInput").ap()
    b_d = nc.dram_tensor("b8", [KT2, 128, B8TOT], F8, kind="ExternalInput").ap()
    c16_d = nc.dram_tensor("c16", [128, KT * 128], F16, kind="ExternalInput").ap()
    d16_d = nc.dram_tensor("d16s", [128, KT * 128], F16, kind="ExternalInput").ap()
    c8_d = nc.dram_tensor("c8", [128, KT2 * 256], F8, kind="ExternalInput").ap()
    m16_d = nc.dram_tensor("mats16", [128, NM16 * 128], BF16, kind="ExternalInput").ap()
    m32_d = nc.dram_tensor("mats32", [128, NM32 * 128], F32, kind="ExternalInput").ap()
    bias_d = nc.dram_tensor("bias", [128, NBIAS], F32, kind="ExternalInput").ap()
    out_d = nc.dram_tensor("out", [1, NSH], F32, kind="ExternalOutput").ap()

    AFT = mybir.ActivationFunctionType

    with tile.TileContext(nc) as tc:
        with (
            tc.tile_pool(name="consts", bufs=1) as consts,
            tc.tile_pool(name="xa", bufs=4) as xa,
            tc.tile_pool(name="xb", bufs=2) as xb,
            tc.tile_pool(name="work", bufs=1) as work,
            tc.tile_pool(name="rt", bufs=4, space="PSUM") as rtp,
            tc.tile_pool(name="pt", bufs=4, space="PSUM") as ptp,
        ):
            # ---- constant tiles (DMAs issued inside the chunk-0 stream) ----
            c16_sb = consts.tile([128, KT, 128], F16, tag="c16")
            d16_sb = consts.tile([128, KT, 128], F16, tag="d16")
            c8_sb = consts.tile([128, KT2 * 2, 128], F8, tag="c8")
            m16_sb = consts.tile([128, NM16 * 128], BF16, tag="m16")
            m32_sb = consts.tile([128, NM32 * 128], F32, tag="m32")
            bias_sb = consts.tile([128, NBIAS], F32, tag="bias")

            def mat16(i):
                return m16_sb[:, i * 128:(i + 1) * 128]

            def bcol(i):
                return bias_sb[:, i:i + 1]

            idt32 = m32_sb[:, M32_IDT * 128:(M32_IDT + 1) * 128]
            idt16 = mat16(M_IDT)
            ones16 = mat16(M_ONES)

            def dma_cd_piece(k):
                """k-th quarter of the c16/d16s constants (19 K-chunks)."""
                t0, t1 = k * 19, min(KT, (k + 1) * 19)
                nc.sync.dma_start(
                    out=c16_sb[:, t0:t1, :],
                    in_=c16_d[:, t0 * 128:t1 * 128].rearrange(
                        "p (t m) -> p t m", m=128))
                nc.sync.dma_start(
                    out=d16_sb[:, t0:t1, :],
                    in_=d16_d[:, t0 * 128:t1 * 128].rearrange(
                        "p (t m) -> p t m", m=128))

            def router_thunks(c):
                """DMA + matmul thunk list for chunk c's router passes."""
                lo, w, w8, boff = LOS[c], WIDTHS[c], W8S[c], B8OFF[c]
                sl = slice(lo, lo + w)
                main_ps = rtp.tile([128, w], F32, tag="rt", name=f"main{c}")
                aux_ps = rtp.tile([128, w], F32, tag="rt", name=f"aux{c}")
                thunks = []
                atiles = [None] * NQ
                btiles = [None, None]

                def dma_a(q):
                    at = xa.tile([128, KQ, w], F16, tag="a", name=f"a{c}_{q}")
                    nc.sync.dma_start(
                        out=at,
                        in_=a_d[q * KQ * 128:(q + 1) * KQ * 128, sl].rearrange(
                            "(t p) j -> p t j", p=128))
                    atiles[q] = at

                def dma_b(hh):
                    t2n = KT2 // 2
                    bt = xb.tile([128, t2n, 2 * w8], F8, tag="b", name=f"b{c}_{hh}")
                    nc.sync.dma_start(
                        out=bt,
                        in_=b_d[hh * t2n:(hh + 1) * t2n, :,
                                boff:boff + 2 * w8].rearrange("t p m -> p t m"))
                    btiles[hh] = bt

                for q in range(NQ):
                    def pre(q=q):
                        dma_a(q)
                        if c == 0:
                            if q < 4:
                                dma_cd_piece(q)
                            if q == 3:
                                nc.sync.dma_start(
                                    out=c8_sb,
                                    in_=c8_d.rearrange("p (t m) -> p t m", m=128))
                                dma_b(0)
                            elif q == 4:
                                dma_b(1)
                                nc.sync.dma_start(out=m16_sb, in_=m16_d)
                                nc.sync.dma_start(out=m32_sb, in_=m32_d)
                                nc.sync.dma_start(out=bias_sb, in_=bias_d)
                        else:
                            if q == 3:
                                dma_b(0)
                            elif q == 4:
                                dma_b(1)

                    for t in range(KQ):
                        def mm_main(q=q, t=t, pre=(pre if t == 0 else None)):
                            if pre:
                                pre()
                            tg = q * KQ + t
                            nc.tensor.matmul(main_ps[:, :], lhsT=c16_sb[:, tg, :],
                                             rhs=atiles[q][:, t, :],
                                             start=(tg == 0), stop=(tg == KT - 1))
                        thunks.append(mm_main)
                    for t in range(KQ):
                        def mm_aux(q=q, t=t):
                            tg = q * KQ + t
                            nc.tensor.matmul(aux_ps[:, :], lhsT=d16_sb[:, tg, :],
                                             rhs=atiles[q][:, t, :],
                                             start=(tg == 0), stop=False)
                        thunks.append(mm_aux)

                if USE_DOUBLE_ROW:
                    for g in range(KT2):
                        def mm_b(g=g):
                            hh, t2 = divmod(g, KT2 // 2)
                            rhs = btiles[hh][:, t2, :].rearrange(
                                "p (s j) -> p s j", s=2)[:, :, 0:w]
                            nc.tensor.matmul(
                                aux_ps[:, :], lhsT=c8_sb[:, 2 * g:2 * g + 2, :],
                                rhs=rhs, start=False, stop=(g == KT2 - 1),
                                perf_mode=mybir.MatmulPerfMode.DoubleRow)
                        thunks.append(mm_b)
                else:
                    for g in range(KT2):
                        for s in range(2):
                            def mm_b(g=g, s=s):
                                hh, t2 = divmod(g, KT2 // 2)
                                nc.tensor.matmul(
                                    aux_ps[:, :], lhsT=c8_sb[:, 2 * g + s, :],
                                    rhs=btiles[hh][:, t2, s * w8:s * w8 + w],
                                    start=False,
                                    stop=(g == KT2 - 1 and s == 1))
                            thunks.append(mm_b)
                return thunks, main_ps, aux_ps

            def post_groups(c, main_ps, aux_ps):
                """Post chain for chunk c as (frac, thunk) groups.

                frac positions the group inside chunk c+1's router matmul
                stream; PE members' dependencies are produced well before
                the PE reaches them, so the tensor engine never stalls.
                """
                lo, w = LOS[c], WIDTHS[c]
                blks = [(off, min(128, w - off)) for off in range(0, w, 128)]
                st = {}

                def g_h():
                    aux_sb = work.tile([128, w], F32, tag="auxs", name="auxs")
                    nc.scalar.activation(aux_sb, aux_ps[:, :], AFT.Identity,
                                         scale=1.0 / RS)
                    st["h"] = work.tile([128, w], F32, tag="h", name="h")
                    nc.vector.tensor_add(st["h"], main_ps[:, :], aux_sb)
                    st["h16"] = work.tile([128, w], BF16, tag="h16", name="h16")
                    nc.scalar.activation(st["h16"], st["h"], AFT.Copy)

                def g_top2():
                    # per-row top-2 threshold + mask, in transposed space
                    st["trs"] = []
                    for blk, (off, cs) in enumerate(blks):
                        tr = ptp.tile([128, 128], F32, tag="pt", name=f"tr{c}_{blk}")
                        nc.tensor.transpose(tr[:cs, :GE], st["h"][:, off:off + cs],
                                            idt32)
                        mx1 = work.tile([128, 1], F32, tag="mx1", name="mx1")
                        nc.vector.reduce_max(mx1[:cs], tr[:cs, :GE],
                                             axis=mybir.AxisListType.X)
                        eqm = work.tile([128, GE], F32, tag="eqm", name="eqm")
                        nc.vector.tensor_scalar(eqm[:cs], tr[:cs, :GE], mx1[:cs],
                                                None, op0=mybir.AluOpType.is_ge)
                        hm = work.tile([128, GE], F32, tag="hm", name="hm")
                        nc.vector.scalar_tensor_tensor(
                            hm[:cs], in0=eqm[:cs], scalar=-1e30, in1=tr[:cs, :GE],
                            op0=mybir.AluOpType.mult, op1=mybir.AluOpType.add)
                        mx2 = work.tile([128, 1], F32, tag="mx2", name="mx2")
                        nc.vector.reduce_max(mx2[:cs], hm[:cs],
                                             axis=mybir.AxisListType.X)
                        mtr = work.tile([128, GE], BF16, tag="mtr", name="mtr")
                        nc.vector.tensor_scalar(mtr[:cs], tr[:cs, :GE], mx2[:cs],
                                                None, op0=mybir.AluOpType.is_ge)
                        st["trs"].append((mtr, off, cs))

                def g_maskback():
                    st["mask_ps"] = ptp.tile([128, w], BF16, tag="pt",
                                             name=f"maskps{c}")
                    for mtr, off, cs in st["trs"]:
                        nc.tensor.transpose(st["mask_ps"][:GE, off:off + cs],
                                            mtr[:cs, :GE], idt16[:cs, :cs])

                def g_gate():
                    eh16 = work.tile([128, w], BF16, tag="eh", name="eh")
                    nc.scalar.activation(eh16, st["h"], AFT.Exp)
                    st["m1"] = work.tile([128, w], BF16, tag="m1", name="m1")
                    nc.vector.tensor_mul(st["m1"], eh16, st["mask_ps"][:, :])

                def g_eo():
                    eo_ps = ptp.tile([128, w], F32, tag="pt", name=f"eo{c}")
                    nc.tensor.matmul(eo_ps[:, :], lhsT=mat16(M_WET), rhs=st["h16"],
                                     start=True, stop=True)
                    st["eo16"] = work.tile([128, w], BF16, tag="eo", name="eo")
                    nc.scalar.activation(st["eo16"], eo_ps[:, :], AFT.Identity,
                                         bias=bcol(B_BE), scale=1.0)

                def g_q():
                    q_ps = ptp.tile([128, w], F32, tag="pt", name=f"q{c}")
                    nc.tensor.matmul(q_ps[:, :], lhsT=mat16(M_AQ), rhs=st["eo16"],
                                     start=True, stop=True)
                    st["qt16"] = work.tile([128, w], BF16, tag="qt", name="qt")
                    nc.scalar.activation(st["qt16"], q_ps[:, :], AFT.Identity,
                                         bias=bcol(B_BQ), scale=1.0)
                    st["sc_ps"] = ptp.tile([128, w], F32, tag="pt", name=f"sc{c}")

                def g_kr(e):
                    kr_ps = ptp.tile([128, w], F32, tag="pt", name=f"kr{c}_{e}")
                    nc.tensor.matmul(kr_ps[:, :], lhsT=mat16(M_AK0 + e),
                                     rhs=st["eo16"], start=True, stop=True)
                    kr16 = work.tile([128, w], BF16, tag="kr", name="kr")
                    nc.scalar.activation(kr16, kr_ps[:, :], AFT.Identity,
                                         bias=bcol(B_BK0 + e), scale=1.0)
                    pe16 = work.tile([128, w], BF16, tag=f"pe{e}", name=f"pe{e}")
                    nc.vector.tensor_mul(pe16, st["qt16"], kr16)
                    st[f"pe{e}"] = pe16

                def g_ms(e):
                    nc.tensor.matmul(st["sc_ps"][:, :], lhsT=mat16(M_MS0 + e),
                                     rhs=st[f"pe{e}"], start=(e == 0),
                                     stop=(e == DH - 1))
                    if e == DH - 1:
                        st["es16"] = work.tile([128, w], BF16, tag="es", name="es")
                        nc.scalar.activation(st["es16"], st["sc_ps"][:, :],
                                             AFT.Exp, scale=0.5)

                def g_den():
                    den_ps = ptp.tile([128, w], F32, tag="pt", name=f"den{c}")
                    nc.tensor.matmul(den_ps[:, :], lhsT=mat16(M_MDEN),
                                     rhs=st["es16"], start=True, stop=True)
                    st["drec"] = work.tile([128, w], F32, tag="drec", name="drec")
                    nc.vector.reciprocal(st["drec"], den_ps[:, :])

                def g_vr(e):
                    vr_ps = ptp.tile([128, w], F32, tag="pt", name=f"vr{c}_{e}")
                    nc.tensor.matmul(vr_ps[:, :], lhsT=mat16(M_AV0 + e),
                                     rhs=st["eo16"], start=True, stop=True)
                    vr16 = work.tile([128, w], BF16, tag="vr", name="vr")
                    nc.scalar.activation(vr16, vr_ps[:, :], AFT.Identity,
                                         bias=bcol(B_BV0 + e), scale=1.0)
                    er_ps = ptp.tile([128, w], F32, tag="pt", name=f"er{c}_{e}")
                    nc.tensor.matmul(er_ps[:, :], lhsT=mat16(M_MER0 + e),
                                     rhs=st["es16"], start=True, stop=True)
                    pr16 = work.tile([128, w], BF16, tag=f"pr{e}", name=f"pr{e}")
                    nc.vector.tensor_mul(pr16, er_ps[:, :], vr16)
                    st[f"pr{e}"] = pr16

                def g_att():
                    s1 = work.tile([128, w], BF16, tag="s1", name="s1")
                    nc.vector.tensor_add(s1, st["pr0"], st["pr1"])
                    s2 = work.tile([128, w], BF16, tag="s2", name="s2")
                    nc.vector.tensor_add(s2, st["pr2"], st["pr3"])
                    s3 = work.tile([128, w], BF16, tag="s3", name="s3")
                    nc.vector.tensor_add(s3, s1, s2)
                    st["att16"] = work.tile([128, w], BF16, tag="att", name="att")
                    nc.vector.tensor_mul(st["att16"], s3, st["drec"])

                def g_ao():
                    ao_ps = ptp.tile([128, w], F32, tag="pt", name=f"ao{c}")
                    nc.tensor.matmul(ao_ps[:, :], lhsT=mat16(M_AO),
                                     rhs=st["att16"], start=True, stop=True)
                    aout16 = work.tile([128, w], BF16, tag="aout", name="aout")
                    nc.scalar.activation(aout16, ao_ps[:, :], AFT.Identity,
                                         bias=bcol(B_BO), scale=1.0)
                    st["num16"] = work.tile([128, w], BF16, tag="num", name="num")
                    nc.vector.tensor_mul(st["num16"], st["m1"], aout16)

                def g_fin():
                    dens_ps = ptp.tile([1, w], F32, tag="pt", name=f"dens{c}")
                    nc.tensor.matmul(dens_ps[:, :], lhsT=ones16[:, 0:1],
                                     rhs=st["m1"], start=True, stop=True)
                    nums_ps = ptp.tile([1, w], F32, tag="pt", name=f"nums{c}")
                    nc.tensor.matmul(nums_ps[:, :], lhsT=ones16[:, 0:1],
                                     rhs=st["num16"], start=True, stop=True)
                    rden = work.tile([1, w], F32, tag="rden", name="rden")
                    nc.vector.reciprocal(rden, dens_ps[:, :])
                    pred = work.tile([1, w], F32, tag="pred", name="pred", bufs=2)
                    nc.vector.tensor_mul(pred, nums_ps[:, :], rden)
                    nc.scalar.dma_start(out=out_d[0:1, lo:lo + w], in_=pred)

                return [
                    (0.00, g_h),
                    (0.06, g_top2),
                    (0.17, g_maskback),
                    (0.21, g_gate),
                    (0.25, g_eo),
                    (0.31, g_q),
                    (0.35, lambda: g_kr(0)),
                    (0.40, lambda: (g_ms(0), g_kr(1))),
                    (0.45, lambda: (g_ms(1), g_kr(2))),
                    (0.50, lambda: (g_ms(2), g_kr(3))),
                    (0.55, lambda: g_ms(3)),
                    (0.61, lambda: (g_den(), g_vr(0))),
                    (0.66, lambda: g_vr(1)),
                    (0.71, lambda: g_vr(2)),
                    (0.76, lambda: g_vr(3)),
                    (0.80, g_att),
                    (0.85, g_ao),
                    (0.92, g_fin),
                ]

            pending = None
            for c in range(NCH):
                thunks, main_ps, aux_ps = router_thunks(c)
                nmm = len(thunks)
                sched = {}
                if pending is not None:
                    for frac, fn in pending:
                        sched.setdefault(min(nmm - 1, int(frac * nmm)), []).append(fn)
                for i, t in enumerate(thunks):
                    t()
                    for fn in sched.get(i, ()):
                        fn()
                pending = post_groups(c, main_ps, aux_ps)
            for frac, fn in pending:
                fn()

    nc.compile()
    return nc


_NC_CACHE = None
LAST_RESULTS = None


def kernel(x, Wr, br, We, be, Wq, bq, Wk, bk, Wv, bv, Wo, bo):
    global _NC_CACHE, LAST_RESULTS
    f32 = np.float32
    x = np.asarray(x, f32)

    c16, d16s, c8, mats16, mats32, biasp = build_consts(
        Wr, br, We, be, Wq, bq, Wk, bk, Wv, bv, Wo, bo)

    if _NC_CACHE is None:
        _NC_CACHE = build_kernel()
    nc = _NC_CACHE

    in_maps = []
    for c in range(NCORES):
        xs = x[c * NSH:(c + 1) * NSH].reshape(NSH, TD)
        a16, b8 = prep_x_shard(xs)
        in_maps.append({"a16": a16, "b8": b8, "c16": c16, "d16s": d16s,
                        "c8": c8, "mats16": mats16, "mats32": mats32,
                        "bias": biasp})

    res = run_bass_kernel_spmd(nc, in_maps, list(range(NCORES)))
    LAST_RESULTS = res
    out = np.concatenate([res.results[c]["out"].reshape(NSH)
                          for c in range(NCORES)])
    return out.astype(f32)
